# revision 10
# baseline (speedup 1.0000x reference)
"""Trainium2 Bass kernel for nn_CAModel (neural cellular automaton step).

Computation (per image, fp32):
  pre_life = maxpool3x3(x[...,3]) > 0.1
  gx, gy   = depthwise 3x3 sobel convs of x
  perc     = interleave([x, gx, gy])            # [H,W,48]
  h        = relu(perc @ w0)                    # [H,W,128]
  dx       = h @ w1                             # [H,W,16]
  x_mid    = x + dx * (update_rand <= 0.5)
  life     = pre_life & (maxpool3x3(x_mid[...,3]) > 0.1)
  x_new    = x_mid * life
  returns (x_new, dx)

Mapping: 8 NeuronCores, pure data-parallel over batch (2 images/core).
The 3x3 conv + fc0 are fused into two matmuls (K=96 covering horizontal
taps 0+1 via a column-shifted duplicate of x on partitions 0-47, plus
K=48 for tap 2) with sobel coefficients folded into the weights on the
host.  fc1 runs operand-swapped (h stationary) so dx lands pixel-major.
The elementwise tail runs in a "pixel slab" layout [128 = w%128,
(img,row,half), c]; maxpool uses free-dim shifts vertically and PE
shift-permutation matmuls horizontally.
"""

import functools
import os
import sys

import numpy as np

_TRN_REPO = os.environ.get("TRN_RL_REPO", "/opt/trn_rl_repo")
if _TRN_REPO not in sys.path:
    sys.path.insert(0, _TRN_REPO)

import concourse.bass as bass
import concourse.bacc as bacc
import concourse.tile as tile
from concourse import mybir
from concourse.bass_utils import run_bass_kernel_spmd

F32 = mybir.dt.float32
F32R = mybir.dt.float32r
BF16 = mybir.dt.bfloat16
BF16_NP = mybir.dt.np(mybir.dt.bfloat16)

C = 16          # channels
HID = 128       # hidden dim
PW = 128        # partitions used as w-position within a half
N_CORES = 8
FIRE_RATE = 0.5
ALIVE_THR = 0.1

LAST_RESULTS = None  # BassKernelResults of the most recent kernel() call


# ---------------------------------------------------------------------------
# device program
# ---------------------------------------------------------------------------

def build_program(NI, H, W, TR=16):
    """Build the Bass program for one core processing NI images of HxW."""
    NH = W // PW                  # halves per row
    assert W % PW == 0 and H % TR == 0 and TR % 2 == 0
    Hp, Wp = H + 2, W + 2
    NRH = NI * H * NH             # total (img,row,half) count
    GRP = TR * NH * C             # psum free size per row-tile (=512 for TR=16)
    assert GRP <= 512

    nc = bacc.Bacc(trn_type="TRN2")

    xh = nc.dram_tensor("xh", [NI * Hp, C, Wp], BF16, kind="ExternalInput")
    x_px = nc.dram_tensor("x_px", [PW, NRH, C], F32, kind="ExternalInput")
    ur = nc.dram_tensor("ur", [PW, NRH], F32, kind="ExternalInput")
    B2d = nc.dram_tensor("B2d", [96, HID], BF16, kind="ExternalInput")
    B3d = nc.dram_tensor("B3d", [48, HID], BF16, kind="ExternalInput")
    w1d = nc.dram_tensor("w1d", [HID, C], BF16, kind="ExternalInput")
    SEd = nc.dram_tensor("SEd", [PW, PW], F32, kind="ExternalInput")
    SWd = nc.dram_tensor("SWd", [PW, PW], F32, kind="ExternalInput")
    dxo = nc.dram_tensor("dxo", [PW, NRH, C], F32, kind="ExternalOutput")
    xno = nc.dram_tensor("xno", [PW, NRH, C], F32, kind="ExternalOutput")

    with tile.TileContext(nc) as tc:
        _emit(tc, locals())
    nc.compile()
    return nc


def _emit(tc, t):
    nc = tc.nc
    NI, H, W, TR = t["NI"], t["H"], t["W"], t["TR"]
    NH, Hp, Wp, NRH, GRP = t["NH"], t["Hp"], t["Wp"], t["NRH"], t["GRP"]
    xh, x_px, ur, B2d, B3d, w1d, SEd, SWd, dxo, xno = (
        t["xh"], t["x_px"], t["ur"], t["B2d"], t["B3d"], t["w1d"], t["SEd"],
        t["SWd"], t["dxo"], t["xno"])
    TRNH = TR * NH
    AL = mybir.AluOpType

    from contextlib import ExitStack
    ctx = ExitStack()
    with ctx:
        singles = ctx.enter_context(tc.tile_pool(name="singles", bufs=1))
        xc_pool = ctx.enter_context(tc.tile_pool(name="xc", bufs=3))
        h_pool = ctx.enter_context(tc.tile_pool(name="h", bufs=8))
        st_pool = ctx.enter_context(tc.tile_pool(name="st", bufs=3))
        ps_h = ctx.enter_context(tc.tile_pool(name="ps_h", bufs=3, space="PSUM"))
        ps_dx = ctx.enter_context(tc.tile_pool(name="ps_dx", bufs=2, space="PSUM"))
        ps_scr = ctx.enter_context(tc.tile_pool(name="ps_scr", bufs=1, space="PSUM"))

        # ---- constants / weights ----
        # b2_sb rows 0-47 = tap dx=1 weights, rows 48-95 = tap dx=0 (matches
        # xc2's partition layout: shifted-copy block first).
        b2_sb = singles.tile([96, HID], BF16)
        nc.sync.dma_start(out=b2_sb, in_=B2d.ap())
        b3_sb = singles.tile([48, HID], BF16)
        nc.sync.dma_start(out=b3_sb, in_=B3d.ap())
        w1_sb = singles.tile([HID, C], BF16)
        nc.sync.dma_start(out=w1_sb, in_=w1d.ap())
        se_sb = singles.tile([PW, PW], F32)
        nc.sync.dma_start(out=se_sb, in_=SEd.ap())
        sw_sb = singles.tile([PW, PW], F32)
        nc.sync.dma_start(out=sw_sb, in_=SWd.ap())

        # ---- update mask ----
        ur_sb = singles.tile([PW, NRH], F32)
        nc.sync.dma_start(out=ur_sb, in_=ur.ap())
        um_sb = singles.tile([PW, NRH], F32)
        nc.vector.tensor_scalar(
            out=um_sb, in0=ur_sb, scalar1=FIRE_RATE, scalar2=None, op0=AL.is_le)

        # ---- PE pre-sync dummies ----
        # Fused 4-byte-weight matmuls (f32/f32r) can carry only one sync
        # wait; touch each DMA-loaded operand once from PE so real matmuls
        # never need more than one.
        scr = ps_scr.tile([PW, 2], F32, tag="scr")
        nc.tensor.matmul(out=scr, lhsT=b2_sb[:], rhs=b2_sb[:, 0:2],
                         start=True, stop=True)
        nc.tensor.matmul(out=scr, lhsT=b3_sb[:], rhs=b3_sb[:, 0:2],
                         start=True, stop=True)
        scr2f = ps_scr.tile([PW, 2], F32, tag="scr")
        scr2 = scr2f[0:C, :]
        nc.tensor.matmul(out=scr2, lhsT=w1_sb[:], rhs=w1_sb[:, 0:2],
                         start=True, stop=True)
        nc.tensor.matmul(out=scr, lhsT=se_sb[:], rhs=se_sb[:, 0:2],
                         start=True, stop=True)
        nc.tensor.matmul(out=scr, lhsT=sw_sb[:], rhs=sw_sb[:, 0:2],
                         start=True, stop=True)

        # ---- residents ----
        xmid = singles.tile([PW, NRH, C], F32)
        ax = singles.tile([PW, NRH], F32)    # alpha of x
        am = singles.tile([PW, NRH], F32)    # alpha of x_mid

        # ---- mask scratch (shared across images) ----
        vm = singles.tile([PW, NRH], F32)
        vm2 = singles.tile([PW, NRH], F32)
        m3 = singles.tile([PW, NRH], F32)
        seam = singles.tile([PW, max(1, 2 * NI * (NH - 1)) * H], F32)
        nc.vector.memset(seam, 0.0)
        plx = singles.tile([PW, NRH], F32)
        plm = singles.tile([PW, NRH], F32)

        def emit_masks_and_xnew(img):
            S, E = img * H * NH, (img + 1) * H * NH

            def half_slice(tile_, p0, cnt, hf):
                return tile_[p0:p0 + cnt, S:E].rearrange(
                    "p (r h) -> p r h", h=NH)[:, :, hf]

            for alpha, pl in ((ax, plx), (am, plm)):
                # vertical 3-max (clamped at image rows); row step in rh = NH
                nc.vector.tensor_tensor(
                    out=vm[:, S + NH:E], in0=alpha[:, S + NH:E],
                    in1=alpha[:, S:E - NH], op=AL.max)
                nc.vector.tensor_copy(out=vm[:, S:S + NH], in_=alpha[:, S:S + NH])
                nc.vector.tensor_tensor(
                    out=vm2[:, S:E - NH], in0=vm[:, S:E - NH],
                    in1=alpha[:, S + NH:E], op=AL.max)
                nc.vector.tensor_copy(out=vm2[:, E - NH:E], in_=vm[:, E - NH:E])

                # horizontal 3-max: PE shift perms + seam fixes
                for s0 in range(S, E, 512):
                    seg = min(512, E - s0)
                    sl = slice(s0, s0 + seg)
                    pse = ps_h.tile([PW, seg], F32, tag="psh")
                    nc.tensor.matmul(out=pse[:, 0:2], lhsT=se_sb[:],
                                     rhs=se_sb[:, 0:2], start=True, stop=True)
                    nc.tensor.matmul(out=pse, lhsT=se_sb[:], rhs=vm2[:, sl],
                                     start=True, stop=True)
                    psw = ps_h.tile([PW, seg], F32, tag="psh")
                    nc.tensor.matmul(out=psw[:, 0:2], lhsT=sw_sb[:],
                                     rhs=sw_sb[:, 0:2], start=True, stop=True)
                    nc.tensor.matmul(out=psw, lhsT=sw_sb[:], rhs=vm2[:, sl],
                                     start=True, stop=True)
                    nc.vector.tensor_tensor(out=m3[:, sl], in0=vm2[:, sl],
                                            in1=pse, op=AL.max)
                    nc.vector.tensor_tensor(out=m3[:, sl], in0=m3[:, sl],
                                            in1=psw, op=AL.max)
                # seam tile is zero except the one partition the DMA fills;
                # max with 0 elsewhere is harmless for the >0.1 threshold.
                for hf in range(NH - 1):
                    # east: p=127 of half hf sees vm2[p=0, half hf+1]
                    be = (img * (NH - 1) + hf) * H
                    nc.sync.dma_start(out=seam[127:128, be:be + H],
                                      in_=half_slice(vm2, 0, 1, hf + 1))
                    nc.vector.tensor_tensor(
                        out=half_slice(m3, 96, 32, hf),
                        in0=half_slice(m3, 96, 32, hf),
                        in1=seam[96:128, be:be + H], op=AL.max)
                    # west: p=0 of half hf+1 sees vm2[p=127, half hf]
                    bw = (NI * (NH - 1) + img * (NH - 1) + hf) * H
                    nc.sync.dma_start(out=seam[0:1, bw:bw + H],
                                      in_=half_slice(vm2, 127, 1, hf))
                    nc.vector.tensor_tensor(
                        out=half_slice(m3, 0, 32, hf + 1),
                        in0=half_slice(m3, 0, 32, hf + 1),
                        in1=seam[0:32, bw:bw + H], op=AL.max)
                nc.vector.tensor_scalar(
                    out=pl[:, S:E], in0=m3[:, S:E], scalar1=ALIVE_THR,
                    scalar2=None, op0=AL.is_gt)

            life = plx  # reuse: life = pre_life * mid_life
            nc.vector.tensor_tensor(out=life[:, S:E], in0=plx[:, S:E],
                                    in1=plm[:, S:E], op=AL.mult)
            for it in range(H // TR):
                g0 = S + it * TRNH
                xns = st_pool.tile([PW, TRNH, C], F32, tag="xns")
                nc.vector.tensor_tensor(
                    out=xns, in0=xmid[:, g0:g0 + TRNH, :],
                    in1=life[:, g0:g0 + TRNH, None].to_broadcast([PW, TRNH, C]),
                    op=AL.mult)
                nc.sync.dma_start(out=xno.ap()[:, g0:g0 + TRNH, :], in_=xns)

        # ================= main matmul + dx/x_mid loop =================
        n_tiles = NI * (H // TR)
        for tt in range(n_tiles):
            img, a = divmod(tt, H // TR)
            a *= TR                       # first real row of this tile
            g0 = (img * H + a) * NH       # first rh index of this tile

            # XC2 partitions 0-47: x shifted one column left (padded col w+1
            # at stored col w); partitions 48-95: unshifted.  Gives taps 0+1
            # as a single K=96 matmul (window 0) and tap 2 as K=48 (window 1
            # into the shifted block) with rhs base partition 0.
            xc2 = xc_pool.tile([96, TR, Wp], BF16)
            src_sh = bass.AP(
                tensor=xh.ap().tensor,
                offset=((img * Hp + a) * C) * Wp + 1,
                ap=[[C * Wp, 3], [Wp, C], [C * Wp, TR], [1, Wp - 1]])
            nc.sync.dma_start(out=xc2[0:48, :, 0:Wp - 1], in_=src_sh)
            src_un = bass.AP(
                tensor=xh.ap().tensor,
                offset=((img * Hp + a) * C) * Wp,
                ap=[[C * Wp, 3], [Wp, C], [C * Wp, TR], [1, Wp]])
            nc.sync.dma_start(out=xc2[48:96, :, :], in_=src_un)

            ps2 = ps_dx.tile([PW, TRNH, C], F32)
            # emit all fc0 matmuls of the tile as one dense PE burst, then
            # the relus, then the fc1 matmuls (LDWEIGHTS-bound)
            pshs, hsbs = [], []
            for p in range(TR // 2):
                psh = ps_h.tile([HID, 2, W], F32)
                pshs.append(psh)
                nc.tensor.matmul(
                    out=psh, lhsT=b2_sb[:],
                    rhs=xc2[0:96, 2 * p:2 * p + 2, 0:W],
                    start=True, stop=False)
                nc.tensor.matmul(
                    out=psh, lhsT=b3_sb[:],
                    rhs=xc2[0:48, 2 * p:2 * p + 2, 1:1 + W],
                    start=False, stop=True)
            for p in range(TR // 2):
                hsb = h_pool.tile([HID, 2, W], BF16)
                hsbs.append(hsb)
                nc.scalar.activation(
                    out=hsb, in_=pshs[p], func=mybir.ActivationFunctionType.Relu)
            for p in range(TR // 2):
                # fc1, operand-swapped: dx[pix, c] chunks
                for rp in range(2):
                    for hf in range(NH):
                        nc.tensor.matmul(
                            out=ps2[:, (2 * p + rp) * NH + hf, :],
                            lhsT=hsbs[p][:, rp, hf * PW:(hf + 1) * PW],
                            rhs=w1_sb[:],
                            start=True, stop=True)

            dxs = st_pool.tile([PW, TRNH, C], F32, tag="dxs")
            nc.vector.tensor_copy(out=dxs, in_=ps2)
            nc.sync.dma_start(out=dxo.ap()[:, g0:g0 + TRNH, :], in_=dxs)

            xps = st_pool.tile([PW, TRNH, C], F32, tag="xps")
            nc.sync.dma_start(out=xps, in_=x_px.ap()[:, g0:g0 + TRNH, :])
            nc.gpsimd.tensor_copy(out=ax[:, g0:g0 + TRNH], in_=xps[:, :, 3])

            # x_mid = x + dx * um
            dxm = st_pool.tile([PW, TRNH, C], F32, tag="dxm")
            nc.vector.tensor_tensor(
                out=dxm, in0=dxs,
                in1=um_sb[:, g0:g0 + TRNH, None].to_broadcast([PW, TRNH, C]),
                op=AL.mult)
            nc.vector.tensor_tensor(
                out=xmid[:, g0:g0 + TRNH, :], in0=xps, in1=dxm, op=AL.add)
            nc.gpsimd.tensor_copy(
                out=am[:, g0:g0 + TRNH], in_=xmid[:, g0:g0 + TRNH, 3])

            if a + TR == H:
                # image finished: its masks + x_new overlap the next
                # image's matmul work
                emit_masks_and_xnew(img)


def _pslice(tile_, p, hf, NH, hrng):
    """[1, hrng] AP of tile_ at partition p, free elements hf::NH."""
    return tile_[p:p + 1, :].rearrange("p (r h) -> p r h", h=NH)[:, :, hf]


def _prange(tile_, p0, cnt, hf, NH, hrng):
    """[cnt, hrng] AP of tile_ at partitions [p0,p0+cnt), free elems hf::NH."""
    return tile_[p0:p0 + cnt, :].rearrange("p (r h) -> p r h", h=NH)[:, :, hf]


# ---------------------------------------------------------------------------
# host side
# ---------------------------------------------------------------------------

def _sobel():
    kx = np.outer([1.0, 2.0, 1.0], [-1.0, 0.0, 1.0]) / 8.0
    ky = kx.T
    return kx, ky


def make_weights(w0, w1):
    """Fold sobel taps into fc0 -> B2[96,128] = [tap1; tap0], B3[48,128]."""
    kx, ky = _sobel()
    w0 = np.asarray(w0, np.float32)         # [48, 128]
    W0x = w0[0::3]                           # [16, 128]
    W0gx = w0[1::3]
    W0gy = w0[2::3]
    Bw = np.zeros((3, 48, HID), np.float32)  # cast to bf16 at return
    for dy in range(3):
        for dxi in range(3):
            m = kx[dy, dxi] * W0gx + ky[dy, dxi] * W0gy
            if dy == 1 and dxi == 1:
                m = m + W0x
            Bw[dxi, dy * C:(dy + 1) * C, :] = m
    B2 = np.concatenate([Bw[1], Bw[0]], axis=0)
    return (B2.astype(BF16_NP), Bw[2].astype(BF16_NP),
            np.asarray(w1, BF16_NP))


def host_inputs(x_core, ur_core, B2, B3, w1, H, W):
    """Build the per-core input map from [NI,H,W,C] x and [NI,H,W,1] rand."""
    NI = x_core.shape[0]
    NH = W // PW
    Hp, Wp = H + 2, W + 2
    xh = np.zeros((NI, Hp, C, Wp), BF16_NP)
    xh[:, 1:H + 1, :, 1:W + 1] = x_core.transpose(0, 1, 3, 2)
    x_px = np.ascontiguousarray(
        x_core.reshape(NI, H, NH, PW, C).transpose(3, 0, 1, 2, 4)
    ).reshape(PW, NI * H * NH, C)
    urp = np.ascontiguousarray(
        ur_core[..., 0].reshape(NI, H, NH, PW).transpose(3, 0, 1, 2)
    ).reshape(PW, NI * H * NH)
    return {
        "xh": xh.reshape(NI * Hp, C, Wp),
        "x_px": x_px,
        "ur": urp,
        "B2d": B2,
        "B3d": B3,
        "w1d": w1,
        "SEd": np.eye(PW, k=-1, dtype=np.float32),
        "SWd": np.eye(PW, k=1, dtype=np.float32),
    }


def unpack_output(dev, NI, H, W):
    """[PW, NRH, C] device layout -> [NI, H, W, C]."""
    NH = W // PW
    return np.ascontiguousarray(
        dev.reshape(PW, NI, H, NH, C).transpose(1, 2, 3, 0, 4)
    ).reshape(NI, H, W, C)


@functools.lru_cache(maxsize=2)
def _cached_program(NI, H, W, TR):
    return build_program(NI, H, W, TR=TR)


def kernel(x, update_rand, w0, w1):
    x = np.asarray(x, np.float32)
    update_rand = np.asarray(update_rand, np.float32)
    B, H, W, _ = x.shape
    NI = B // N_CORES
    B2, B3, w1f = make_weights(w0, w1)

    nc = _cached_program(NI, H, W, 16)
    in_maps = [
        host_inputs(x[i * NI:(i + 1) * NI], update_rand[i * NI:(i + 1) * NI],
                    B2, B3, w1f, H, W)
        for i in range(N_CORES)
    ]
    res = run_bass_kernel_spmd(nc, in_maps, core_ids=list(range(N_CORES)))
    global LAST_RESULTS
    LAST_RESULTS = res
    x_new = np.concatenate(
        [unpack_output(r["xno"], NI, H, W) for r in res.results], axis=0)
    dx = np.concatenate(
        [unpack_output(r["dxo"], NI, H, W) for r in res.results], axis=0)
    return x_new, dx

